# revision 46
# baseline (speedup 1.0000x reference)
"""Trainium2 Bass kernel for diffusers AttnProcessor self-attention.

Reference computation (fp32, B=2, S=4096, C=512, H=8, D=64):
    q = hs @ Wq.T ; k = hs @ Wk.T ; v = hs @ Wv.T          (per-head split)
    probs = softmax(q k^T / sqrt(D))                        [b,h,s,s]
    out = (probs @ v) @ Wo.T + bo                           [b,s,c]

Sharding: 8 cores = (batch b in 0..1) x (query-slice of 1024 rows in 0..3).
Each core holds the full X[b] (for K/V projections) and produces the full
output rows for its query slice -> concatenating core outputs along rows
gives the full [B*S, C] output directly.

Host <-> device dataflow (the axon tunnel in this container moves only
~25-45 MB/s, so wire bytes dominate end-to-end wall time):
  up:   hidden_states as bf16 [B*S, C] row-sharded over the 8 cores (1/8
        each), weights as bf16 row-sharded, bo replicated (2 KB).
  prep: an XLA program (shard_map) all-gathers the full activations over
        NeuronLink, transposes, scales Wq, and emits exactly the per-core
        DRAM tensors the Bass kernel expects (xt/xtq per-core, w*t/bob
        replicated) plus the donated output buffer -- all device-side.
  exec: the Bass NEFF (one dispatch).
  down: out as fp16 [B*S, C] row-sharded (8 MB).

Device dataflow per core (all matmuls bf16 in / fp32 PSUM accum):
  Xt = X[b]^T in SBUF                              [C=512, S=4096]
  Qt = (Wq^T/sqrt(D)) @ Xt_q  per head-pair        [128, 1024]
  Kt = Wk^T @ Xt              per head-pair        [128, 4096]
  (a per-head copy of Qt/Kt rows is DMA'd to the opposite partition half so
   the two sq-chunks of the QK^T matmul run in disjoint PE row groups; the
   copies for head h+1 are prefetched near the end of head h)
  V' = [X @ Wv_pair^T | 1] per pair                [S, 2*65] per pair
  per head h, per key tile t (128 keys):
    St[t] = Kt_h[:,t]^T Qt_h        [128 sk, 1024 sq]  (2 row-packed matmuls)
    Pt    = exp(St)                 (ScalarE, bf16 out)
    O'_h += V'[t]^T Pt              [65, 1024]  (row 64 = softmax denominator)
  O_h = O'_h[0:64] * (1/O'_h[64])   -> Ot (head-concat layout)
  out = Ot^T @ Wo^T + bo            -> DMA out  [1024, 512] fp16

The inner QK->exp->PV pipeline is ACT-bound (exp ~1038ns/tile vs PE
~854ns/tile), so all other PE work -- the next pair's V/K/Q projections and
finished pairs' output-projection chunks -- is drip-fed into the tile slots
via a budgeted task queue (DRAIN_BUDGET ns/slot) instead of clustering at
head starts.

build_nc(loop_n=N) wraps the whole kernel (input loads + compute + output
store) in a hardware For_i loop so one NEFF executes the kernel N times
back-to-back -- used to measure per-execution device time independent of
RPC/transfer overhead.
"""

import numpy as np
import ml_dtypes
from contextlib import ExitStack

import concourse.bass as bass
import concourse.bacc as bacc
import concourse.mybir as mybir
import concourse.tile as tile
from concourse import dve_ops as _dve_ops
from concourse.dve_spec import (
    Spec as _Spec, Src0 as _Src0, C0 as _C0, C1 as _C1, C2 as _C2,
    sq as _sq, lower as _dve_lower, _has_src1,
)
from concourse.dve_uop import DveOpSpec as _DveOpSpec

BF16 = mybir.dt.bfloat16
F16 = mybir.dt.float16
F32 = mybir.dt.float32

B, S, C, H, D = 2, 4096, 512, 8, 64
NCORES = 8
SQ = 1024          # query rows per core
P = 128            # partitions
NSK = S // P       # 32 key tiles
NCI = C // P       # 4 contraction tiles for projections
SQC = 512          # matmul moving free dim
NSQC = SQ // SQC   # 2
E = D + 1          # V' columns per head (64 v cols + ones col)

ROW_PACK = True
DRAIN_BUDGET = 250  # drip-feed rate (ns/slot) for deferred projection tasks
PT_BUFS = 3
DVE_EXP = False    # offload every 4th exp tile from ScalarE to a custom DVE op

# quadratic Chebyshev fit of exp(x/16) on [-2.2, 2.2]; q(x)^16 ~ exp(x)
# (max rel err 0.2% in range; scores here are < +-1.3)
_EXPC = (1.0, 0.06264781, 0.00195543)


def _register_exp16():
    """Register a custom DVE op computing q(x)^16 ~ exp(x) (8 ALU stages)."""
    for op in _dve_ops.OPS:
        if op.name == "EXP16_ANT":
            return op
    q = (_Src0 * _C2 + _C1) * _Src0 + _C0
    spec = _Spec(
        body=_sq(_sq(_sq(_sq(q)))),
        reference=lambda in0, in1, s0, s1, imm2: (
            ((in0 * np.float32(imm2) + np.float32(s1)) * in0 + np.float32(s0))
            ** 16).astype(np.float32),
    )
    idx = max(_dve_ops._SUB_OPCODE_FOR_NAME.values()) + 1
    assert idx < 0x20
    op = _dve_ops.DveOp("EXP16_ANT", spec, subdim=False, uops_sha={})
    _dve_ops.OPS.append(op)
    _dve_ops.CUSTOM_DVE_SPECS[op.name] = spec
    _dve_ops._SUB_OPCODE_FOR_NAME[op.name] = idx
    for ver in ("v3",):
        s = _DveOpSpec(name=op.name, opcode=idx, uops=_dve_lower(spec, ver=ver),
                       rd1_en=_has_src1(spec))
        op.uops_sha[ver] = s.sha(ver)
    return op


EXP16 = _register_exp16()


def build_nc(row_pack=ROW_PACK, reps=1, dve_exp=None, loop_n=1):
    if dve_exp is None:
        dve_exp = DVE_EXP
    nc = bacc.Bacc("TRN2", target_bir_lowering=False, debug=False,
                   num_devices=NCORES)

    xt_d = nc.dram_tensor("xt", [C, S], BF16, kind="ExternalInput").ap()
    xtq_d = nc.dram_tensor("xtq", [C, SQ], BF16, kind="ExternalInput").ap()
    wqt_d = nc.dram_tensor("wqt", [C, C], BF16, kind="ExternalInput").ap()
    wkt_d = nc.dram_tensor("wkt", [C, C], BF16, kind="ExternalInput").ap()
    wvt_d = nc.dram_tensor("wvt", [C, C], BF16, kind="ExternalInput").ap()
    wot_d = nc.dram_tensor("wot", [C, C], BF16, kind="ExternalInput").ap()
    bob_d = nc.dram_tensor("bob", [P, C], F32, kind="ExternalInput").ap()
    out_d = nc.dram_tensor("out", [SQ, C], F16, kind="ExternalOutput").ap()

    with ExitStack() as ctx:
        tc = ctx.enter_context(tile.TileContext(nc))
        const = ctx.enter_context(tc.tile_pool(name="const", bufs=1))
        work = ctx.enter_context(tc.tile_pool(name="work", bufs=2))
        psum = ctx.enter_context(tc.tile_pool(name="psum", bufs=2, space="PSUM"))

        def emit_all():
            def load_tiles(dram_ap, n, cols, dtype, base, eng=None):
                tiles = []
                for ci in range(n):
                    t = const.tile([P, cols], dtype, name=f"{base}{ci}",
                                   tag=f"{base}{ci}")
                    (eng or nc.sync).dma_start(t, dram_ap[ci * P:(ci + 1) * P, :])
                    tiles.append(t)
                return tiles

            # Input loads split between the SP queue and the (startup-idle) ACT
            # queue, ordered by first use; the first QK^T tile needs
            # xtq+wqt+wkt+xt[ck0] only. Dependent SBUF<->SBUF moves go on the
            # gpsimd queue so they can't FIFO-block behind these.
            xtq_sb = load_tiles(xtq_d, NCI, SQ, BF16, "xtqs", eng=nc.scalar)
            wqt_sb = load_tiles(wqt_d, NCI, C, BF16, "wqts", eng=nc.scalar)
            wkt_sb = load_tiles(wkt_d, NCI, C, BF16, "wkts")
            xt_sb = [const.tile([P, S], BF16, name=f"xts{ci}", tag=f"xts{ci}")
                     for ci in range(NCI)]
            for ci in range(NCI):
                nc.sync.dma_start(xt_sb[ci][:, 0:SQC],
                                  xt_d[ci * P:(ci + 1) * P, 0:SQC])
            wvt_sb = load_tiles(wvt_d, NCI, C, BF16, "wvts")
            for ck in range(1, S // SQC):
                for ci in range(NCI):
                    sl = slice(ck * SQC, (ck + 1) * SQC)
                    nc.sync.dma_start(xt_sb[ci][:, sl],
                                      xt_d[ci * P:(ci + 1) * P, sl])
            wot_sb = load_tiles(wot_d, NCI, C, BF16, "wots")
            bob_sb = const.tile([P, C], F32, name="bobs", tag="bobs")
            nc.sync.dma_start(bob_sb, bob_d)
            ones_sb = const.tile([P, D], mybir.dt.float16, name="ones_sb",
                                 tag="ones_sb")
            nc.vector.memset(ones_sb, 1.0)

            for rep in range(reps):
                emit_body(nc, tc, const, work, psum, (row_pack, dve_exp),
                          xt_sb, xtq_sb, wqt_sb, wkt_sb, wvt_sb, wot_sb,
                          bob_sb, ones_sb, out_d)

        if loop_n > 1:
            with tc.For_i(0, loop_n):
                emit_all()
        else:
            emit_all()

    nc.compile()
    return nc


def emit_body(nc, tc, const, work, psum, flags,
              xt_sb, xtq_sb, wqt_sb, wkt_sb, wvt_sb, wot_sb,
              bob_sb, ones_sb, out_d):
    row_pack, dve_exp = flags
    vp_sb = {}                 # (pair, tile) -> [P, 2*E] bf16 tile

    def emit_vproj(p, t_i):
        """Per-pair V projection: only pair p's two heads' V columns for key
        tile t_i (213ns of PE vs 853ns for the old all-heads version), so it
        can be drip-fed into ACT-bound slots of the previous pair."""
        if (p, t_i) in vp_sb:
            return
        vps = psum.tile([P, 2 * D], F32, name="vps", tag="proj")
        for ci in range(NCI):
            nc.tensor.matmul(vps, lhsT=xt_sb[ci][:, t_i * P:(t_i + 1) * P],
                             rhs=wvt_sb[ci][:, p * P:(p + 1) * P],
                             start=(ci == 0), stop=(ci == NCI - 1))
        # Stationary padded to a full 128 columns (V cols | ones | zeros):
        # FWL (fast weight load) only triggers at NumWeights==128, and the
        # PV matmuls dominate the stationary-load count. The extra zero
        # columns write junk-free zeros into oacc rows E..127 (never read)
        # and cost nothing -- matmul duration is N rows regardless of M.
        vp = work.tile([P, 2 * P], BF16, name=f"vp{t_i}", tag=f"vp{t_i}",
                       bufs=2)
        vp3 = vp.rearrange("p (h m) -> p h m", m=P)
        nc.vector.tensor_copy(out=vp3[:, :, 0:D],
                              in_=vps.rearrange("p (h d) -> p h d", d=D))
        nc.vector.memset(vp3[:, :, D:E], 1.0)
        nc.vector.memset(vp3[:, :, E:P], 0.0)
        vp_sb[(p, t_i)] = vp

    def emit_qtp_chunk(qtp, p, cq):
        qps = psum.tile([P, SQC], F32, name="qps", tag="proj")
        for ci in range(NCI):
            nc.tensor.matmul(
                qps, lhsT=wqt_sb[ci][:, p * P:(p + 1) * P],
                rhs=xtq_sb[ci][:, cq * SQC:(cq + 1) * SQC],
                start=(ci == 0), stop=(ci == NCI - 1))
        nc.vector.tensor_copy(out=qtp[:, cq * SQC:(cq + 1) * SQC], in_=qps)

    def emit_ktp_chunk(ktp, p, ck):
        kps = psum.tile([P, SQC], F32, name="kps", tag="proj")
        for ci in range(NCI):
            nc.tensor.matmul(
                kps, lhsT=wkt_sb[ci][:, p * P:(p + 1) * P],
                rhs=xt_sb[ci][:, ck * SQC:(ck + 1) * SQC],
                start=(ci == 0), stop=(ci == NCI - 1))
        nc.vector.tensor_copy(out=ktp[:, ck * SQC:(ck + 1) * SQC], in_=kps)

    # Ot: normalized attention output, head-concat layout [c_in, sq]
    ot_sb = [const.tile([P, SQ], BF16, name=f"ot{i}", tag=f"ot{i}")
             for i in range(NCI)]

    def make_norm_tail(h, oraw, r):
        """Broadcast-matmul + normalize for head h. Deferred into the next
        head's loop so the PE-stream bcast matmul never waits on the DVE
        recip (PE is in-order; an early bcast would bubble the pipeline)."""
        def tail():
            rbp = psum.tile([D, SQ], F32, name="rbp", tag="st")
            for cq in range(NSQC):
                sl = slice(cq * SQC, (cq + 1) * SQC)
                nc.tensor.matmul(rbp[:, sl], lhsT=ones_sb[D:D + 1, :],
                                 rhs=r[D:D + 1, sl], start=True, stop=True)
            rb = work.tile([D, SQ], F32, name="rb", tag="rb", bufs=2)
            nc.vector.tensor_copy(out=rb, in_=rbp)
            if h % 2 == 0:
                nc.vector.tensor_mul(out=ot_sb[h // 2][0:D, :],
                                     in0=oraw[0:D, :], in1=rb)
            else:
                # DVE lanes are partition-locked; move to the upper half by DMA
                otmp = work.tile([D, SQ], BF16, name="otmp", tag="otmp",
                                 bufs=2)
                nc.vector.tensor_mul(out=otmp, in0=oraw[0:D, :], in1=rb)
                nc.gpsimd.dma_start(ot_sb[h // 2][D:2 * D, :], otmp)
        return tail

    outacc = const.tile([P, S], F32, name="outacc", tag="outacc")
    outf16 = const.tile([P, S], F16, name="outf16", tag="outf16")

    def make_oproj_chunk(pair, sqt):
        """One 128-row slice of pair `pair`'s output projection, accumulated
        into outacc (SBUF). Emitted as a drip-fed task so the PE lumps land
        in ACT-bound slots. The final pair writes fp16 (the DRAM out dtype)
        and stores."""
        def fn():
            ops = psum.tile([P, C], F32, name="ops", tag="proj")
            nc.tensor.matmul(ops,
                             lhsT=ot_sb[pair][:, sqt * P:(sqt + 1) * P],
                             rhs=wot_sb[pair], start=True, stop=True)
            osl = outacc[:, sqt * C:(sqt + 1) * C]
            if pair == 0:
                nc.vector.tensor_add(osl, ops, bob_sb)
            elif pair < NCI - 1:
                nc.vector.tensor_add(osl, osl, ops)
            else:
                with nc.allow_low_precision("fp16 output; ~1e-4 rel"):
                    nc.vector.tensor_add(
                        outf16[:, sqt * C:(sqt + 1) * C], osl, ops)
                nc.gpsimd.dma_start(
                    out_d[sqt * P:(sqt + 1) * P, :],
                    outf16[:, sqt * C:(sqt + 1) * C])
        return (500, fn, ("o", pair, sqt))

    # Deferred PE work (next pair's V/K/Q projections, finished pairs'
    # output projections) drained at a budgeted ns/slot rate so it lands in
    # the ACT-bound slots' PE slack instead of clustering in head 0.
    task_q = []                # (cost_ns, fn, done_key)
    done = set()
    credit = [0.0]

    def drain(budget_ns):
        credit[0] = min(credit[0] + budget_ns, 1800.0)
        while task_q and (task_q[0][0] <= credit[0]
                          or budget_ns == float("inf")):
            cost, fn, key = task_q.pop(0)
            if key is None or key not in done:
                fn()
                if key is not None:
                    done.add(key)
                credit[0] -= cost

    pair_st = {}               # pair -> dict(qtp, ktp, kdone, qdone)

    def new_pair(p):
        pair_st[p] = dict(
            qtp=work.tile([P, SQ], BF16, name="qtp", tag="qtp"),
            ktp=work.tile([P, S], BF16, name="ktp", tag="ktp"),
            kdone=0, qdone=0)
        return pair_st[p]

    def task_k(p, ck):
        def fn():
            ps = pair_st[p]
            if ps["kdone"] == ck:      # may have been emitted just-in-time
                emit_ktp_chunk(ps["ktp"], p, ck)
                ps["kdone"] += 1
        return (853, fn, ("k", p, ck))

    def task_q_proj(p, cq):
        def fn():
            ps = pair_st[p]
            if ps["qdone"] == cq:
                emit_qtp_chunk(ps["qtp"], p, cq)
                ps["qdone"] += 1
        return (853, fn, ("q", p, cq))

    def task_v(p, t):
        return (213, lambda: emit_vproj(p, t), ("v", p, t))

    def enqueue_pair_prep(p):
        """Next pair's projections, in next-h0 consumption order."""
        new_pair(p)
        task_q.append(task_q_proj(p, 0))
        task_q.append(task_q_proj(p, 1))
        for t in range(NSK):
            if t % 4 == 0:
                task_q.append(task_k(p, t // 4))
            task_q.append(task_v(p, t))

    pending_norm = None
    swaps = {}
    for h in range(H):
        p, half = h // 2, h % 2
        lo, hi = half * D, half * D + D          # head's rows in pair tiles
        olo, ohi = D - half * D, 2 * D - half * D  # opposite half rows

        if half == 0:
            if p not in pair_st:
                new_pair(p)          # pair 0: everything runs just-in-time
            ps = pair_st[p]
            qtp, ktp = ps["qtp"], ps["ktp"]

        def need_q(cq):
            while ps["qdone"] <= cq:
                emit_qtp_chunk(qtp, p, ps["qdone"])
                done.add(("q", p, ps["qdone"]))
                ps["qdone"] += 1

        def need_k(ck):
            while ps["kdone"] <= ck:
                emit_ktp_chunk(ktp, p, ps["kdone"])
                done.add(("k", p, ps["kdone"]))
                ps["kdone"] += 1

        need_q(NSQC - 1)
        # per-head swap copies: same rows duplicated into the other
        # partition half so both sq-chunks can use disjoint PE row groups.
        # The qts copy + first kts chunk for head hh are prefetched near the
        # end of the previous head (prep_swaps) so head-start QK never waits
        # on the Pool-queue DMA chain.
        def prep_swaps(hh):
            if not row_pack or hh in swaps:
                return
            hp, hhalf = hh // 2, hh % 2
            l0 = hhalf * D
            ol0 = D - hhalf * D
            pst = pair_st.get(hp)
            if pst is None or pst["qdone"] < NSQC or pst["kdone"] < 1:
                return                    # projections not ready yet
            qts = work.tile([P, SQ], BF16, name="qts", tag="qts")
            nc.gpsimd.dma_start(qts[ol0:ol0 + D, :],
                                pst["qtp"][l0:l0 + D, :])
            kts = work.tile([P, S], BF16, name="kts", tag="kts")
            swaps[hh] = dict(qts=qts, kts=kts, done=0)
            swap_chunk(hh, 0)

        def swap_chunk(hh, ck):
            sw = swaps.get(hh)
            if not row_pack or sw is None:
                return
            hp, hhalf = hh // 2, hh % 2
            l0 = hhalf * D
            ol0 = D - hhalf * D
            while sw["done"] <= ck:            # catch up through chunk ck
                c = sw["done"]
                nc.gpsimd.dma_start(
                    sw["kts"][ol0:ol0 + D, c * SQC:(c + 1) * SQC],
                    pair_st[hp]["ktp"][l0:l0 + D, c * SQC:(c + 1) * SQC])
                sw["done"] = c + 1

        need_k(0)
        prep_swaps(h)
        if row_pack:
            qts, kts = swaps[h]["qts"], swaps[h]["kts"]
        oacc = psum.tile([P, SQ], F32, name="oacc", tag="oacc", bufs=1)
        for t_i in range(NSK):
            # swap-copy the next K chunk one window early so the QK matmuls
            # never wait on the projection->evict->swap-DMA chain
            if t_i % 4 == 1 and t_i // 4 + 1 < S // SQC:
                need_k(t_i // 4 + 1)
                swap_chunk(h, t_i // 4 + 1)
            need_k(t_i // 4)
            emit_vproj(p, t_i)
            if t_i == 8 and pending_norm is not None:
                h_prev, tail = pending_norm
                tail()
                pending_norm = None
                if h_prev % 2 == 1:
                    for sqt in range(SQ // P):
                        task_q.append(make_oproj_chunk(h_prev // 2, sqt))
            if t_i == 2 and half == 1 and h + 1 < H:
                enqueue_pair_prep(p + 1)
            if t_i == 28 and h + 1 < H:
                prep_swaps(h + 1)
            drain(DRAIN_BUDGET)

            st = psum.tile([P, SQ], F32, name="st", tag="st", bufs=2)
            ksl = slice(t_i * P, (t_i + 1) * P)
            if row_pack:
                nc.tensor.matmul(st[:, 0:SQC], lhsT=ktp[lo:hi, ksl],
                                 rhs=qtp[lo:hi, 0:SQC],
                                 start=True, stop=True,
                                 tile_position=(lo, 0))
                nc.tensor.matmul(st[:, SQC:SQ], lhsT=kts[olo:ohi, ksl],
                                 rhs=qts[olo:ohi, SQC:SQ],
                                 start=True, stop=True,
                                 tile_position=(olo, 0))
            else:
                for cq in range(NSQC):
                    nc.tensor.matmul(
                        st[:, cq * SQC:(cq + 1) * SQC],
                        lhsT=ktp[lo:hi, ksl],
                        rhs=qtp[lo:hi, cq * SQC:(cq + 1) * SQC],
                        start=True, stop=True)
            pt = work.tile([P, SQ], BF16, name="pt", tag="pt", bufs=PT_BUFS)
            if dve_exp and t_i % dve_exp == dve_exp - 1:
                nc.vector._custom_dve(EXP16, out=pt, in0=st,
                                      s0=_EXPC[0], s1=_EXPC[1], imm2=_EXPC[2])
            else:
                nc.scalar.activation(out=pt, in_=st,
                                     func=mybir.ActivationFunctionType.Exp)
            for cq in range(NSQC):
                nc.tensor.matmul(
                    oacc[:, cq * SQC:(cq + 1) * SQC],
                    lhsT=vp_sb[(p, t_i)][:, half * P:(half + 1) * P],
                    rhs=pt[:, cq * SQC:(cq + 1) * SQC],
                    start=(t_i == 0), stop=(t_i == NSK - 1))

        # evict oacc to SBUF immediately so the PSUM slot frees for the next
        # head; the bcast+normalize runs deferred, off the critical path
        oraw = work.tile([E, SQ], F32, name="oraw", tag="oraw", bufs=2)
        nc.vector.tensor_copy(out=oraw, in_=oacc[0:E, :])
        r = work.tile([E, SQ], mybir.dt.float16, name="r", tag="r", bufs=2)
        with nc.allow_low_precision("softmax denom recip; fp16 ~1e-4 rel"):
            nc.vector.reciprocal(r[D:E, :], oraw[D:E, :])
        pending_norm = (h, make_norm_tail(h, oraw, r))

    drain(float("inf"))                # any remaining deferred work
    pending_norm[1]()                  # final head's normalization
    for sqt in range(SQ // P):         # final pair's projection + store
        make_oproj_chunk(NCI - 1, sqt)[1]()


# ---------------------------------------------------------------------------
# Host-side runner: cached jitted prep (XLA, on-device gather/transpose) +
# cached jitted Bass exec. Built lazily on first kernel() call.
# ---------------------------------------------------------------------------

_STATE = {}


def make_exec_fn(nc, mesh):
    """Jitted shard_map wrapper around the Bass NEFF (mirrors
    bass2jax.run_bass_via_pjrt but reusable across calls).

    Input order: xt, xtq, wqt, wkt, wvt, wot, bob (from BIR allocation
    order), then the donated output buffer. xt/xtq/out are per-core
    (P('core')), the rest replicated (P())."""
    import jax
    from jax.experimental.shard_map import shard_map
    from jax.sharding import PartitionSpec
    from concourse import bass2jax

    bass2jax.install_neuronx_cc_hook()
    partition_name = (nc.partition_id_tensor.name
                      if nc.partition_id_tensor else None)

    in_names, out_names, out_avals = [], [], []
    for alloc in nc.m.functions[0].allocations:
        if not isinstance(alloc, mybir.MemoryLocationSet):
            continue
        name = alloc.memorylocations[0].name
        if alloc.kind == "ExternalInput":
            if name != partition_name:
                in_names.append(name)
        elif alloc.kind == "ExternalOutput":
            out_names.append(name)
            out_avals.append(jax.core.ShapedArray(
                tuple(alloc.tensor_shape), mybir.dt.np(alloc.dtype)))
    n_params = len(in_names)
    all_in_names = tuple(in_names + out_names)
    if partition_name is not None:
        all_in_names = all_in_names + (partition_name,)
    donate = tuple(range(n_params, n_params + len(out_names)))

    def _body(*args):
        operands = list(args)
        if partition_name is not None:
            operands.append(bass2jax.partition_id_tensor())
        return tuple(bass2jax._bass_exec_p.bind(
            *operands,
            out_avals=tuple(out_avals),
            in_names=all_in_names,
            out_names=tuple(out_names),
            lowering_input_output_aliases=(),
            sim_require_finite=True,
            sim_require_nnan=True,
            nc=nc,
        ))

    spec_of = {"xt": PartitionSpec("core"), "xtq": PartitionSpec("core"),
               "wqt": PartitionSpec(), "wkt": PartitionSpec(),
               "wvt": PartitionSpec(), "wot": PartitionSpec(),
               "bob": PartitionSpec()}
    in_specs = tuple(spec_of[n] for n in in_names) + \
        (PartitionSpec("core"),) * len(out_names)
    out_specs = (PartitionSpec("core"),) * len(out_names)
    fn = jax.jit(
        shard_map(_body, mesh=mesh, in_specs=in_specs, out_specs=out_specs,
                  check_rep=False),
        donate_argnums=donate, keep_unused=True)
    return fn, in_names


def make_prep_fn(mesh):
    """Jitted XLA prep: all-gather sharded host uploads on-device, build the
    exact DRAM tensors the Bass kernel wants + the donated out buffer."""
    import jax
    import jax.numpy as jnp
    from jax.experimental.shard_map import shard_map
    from jax.sharding import PartitionSpec

    scale = np.float32(D) ** -0.5

    def _prep(hb, wq, wk, wv, wo, bo):
        # hb: local [B*S/8, C] bf16 shard of the row-major activations
        full = jax.lax.all_gather(hb, "core", axis=0, tiled=True)  # [B*S, C]
        xt_all = full.T                                            # [C, B*S]
        idx = jax.lax.axis_index("core")
        b = idx // (NCORES // B)
        xt = jax.lax.dynamic_slice(xt_all, (0, b * S), (C, S))
        q0 = b * S + (idx % (NCORES // B)) * SQ
        xtq = jax.lax.dynamic_slice(xt_all, (0, q0), (C, SQ))

        def wt(w):
            return jax.lax.all_gather(w, "core", axis=0, tiled=True).T

        wqt = wt(wq) * jnp.bfloat16(scale)
        wkt, wvt, wot = wt(wk), wt(wv), wt(wo)
        bob = jnp.broadcast_to(bo[None, :], (P, C))
        zeros = jnp.zeros((SQ, C), jnp.float16)
        return xt, xtq, wqt, wkt, wvt, wot, bob, zeros

    core, repl = PartitionSpec("core"), PartitionSpec()
    return jax.jit(shard_map(
        _prep, mesh=mesh,
        in_specs=(core, core, core, core, core, repl),
        out_specs=(core, core, repl, repl, repl, repl, repl, core),
        check_rep=False))


def _get_state():
    if "exec" not in _STATE:
        import jax
        from jax.sharding import Mesh
        mesh = Mesh(np.asarray(jax.devices()[:NCORES]), ("core",))
        nc = build_nc()
        exec_fn, in_names = make_exec_fn(nc, mesh)
        assert in_names == ["xt", "xtq", "wqt", "wkt", "wvt", "wot", "bob"], \
            in_names  # prep_fn's return order must match
        _STATE.update(mesh=mesh, nc=nc, exec=exec_fn, in_names=in_names,
                      prep=make_prep_fn(mesh))
    return _STATE


def host_prep(hidden_states, Wq, Wk, Wv, Wo, bo):
    """Cheap host-side casts; all layout work happens on device."""
    bf16 = ml_dtypes.bfloat16
    hb = np.asarray(hidden_states, np.float32).reshape(B * S, C).astype(bf16)
    return (hb, np.asarray(Wq, np.float32).astype(bf16),
            np.asarray(Wk, np.float32).astype(bf16),
            np.asarray(Wv, np.float32).astype(bf16),
            np.asarray(Wo, np.float32).astype(bf16),
            np.asarray(bo, np.float32))


def device_inputs(inputs):
    """Upload + on-device prep; returns the exec_fn argument list (device
    arrays: xt, xtq, wqt, wkt, wvt, wot, bob, out-buffer)."""
    st = _get_state()
    return st["prep"](*host_prep(**inputs))


def run(inputs):
    """Full pipeline: host cast -> device prep -> Bass exec -> fetch.
    Returns the [B, S, C] fp32 output."""
    import jax
    st = _get_state()
    dev = device_inputs(inputs)
    outs = st["exec"](*dev)
    out = np.asarray(outs[0]).astype(np.float32)   # [B*S, C]
    return out.reshape(B, S, C)


def kernel(**inputs):
    return run(inputs)
